# revision 70
# baseline (speedup 1.0000x reference)
"""Trainium2 Bass kernel for nn_Attention_79224966742132.

Dense transformer attention block: QKV projection + axial RoPE + SDPA +
output projection, for x (2, 2048, 1152), 16 heads of dim 72.

Sharding (8 cores): data-parallel over batch (2) x tensor-parallel over
head groups (4 heads/core). Each core computes QKV for its 4 heads from
the full x[b], applies RoPE, runs attention, and produces a partial
output projection (row-parallel Wproj); the host sums the 4 partials per
batch element. The projection bias rides on the g==0 core of each batch.

Engine plan (per the TRN2 cost model):
 - all GEMM inputs bf16 (1 cycle/row); scores q/k quantized to fp8e4 and
   run in DoubleRow perf mode (0.5 cycles/row) with the 72-dim
   contraction packed as [36 partitions x 2 slots].
 - exp of all N^2 scores is the critical ~130us ACT chain; score->exp
   units are interleaved into phase 1 (capped by the e_t inventory E so
   the in-order PE queue can never block on a PV-gated slot) so ACT
   starts early and never runs dry afterwards.
 - PV is "flipped": out [128 q, 73] with the exp tile as stationary, so
   each matmul streams only F=73 rows; a ones column in V yields the
   softmax denominator; normalize via per-partition tensor_scalar; PE
   transposes (bf16) restore o^T for the row-parallel projection.
 - projection contraction packed to [128,128,33] chunks with the bias as
   a ones-row in the last chunk (o^T rows staged via aligned DVE copy +
   partition-arbitrary SBUF->SBUF DMA).
"""
import math
import os
import sys

# The device path needs the axon/neuron jax platform; if a harness pinned
# JAX_PLATFORMS=cpu (common for running jax references) and jax is not yet
# imported, restore platform auto-detection.
if "jax" not in sys.modules:
    _jp = os.environ.get("JAX_PLATFORMS")
    if _jp and "axon" not in _jp and "neuron" not in _jp:
        del os.environ["JAX_PLATFORMS"]

import numpy as np
import ml_dtypes

import bass_rust
import concourse.bass as bass
import concourse.mybir as mybir
import concourse.tile as tile
from concourse.bass_utils import run_bass_kernel_spmd

F32 = mybir.dt.float32
BF16 = mybir.dt.bfloat16
F8 = mybir.dt.float8e4
AF = mybir.ActivationFunctionType
ALU = mybir.AluOpType
DR = mybir.MatmulPerfMode.DoubleRow

B = 2
N = 2048          # tokens = T*H*W = 8*16*16
C = 1152
NH = 16
HD = 72
HPG = 4           # heads per core
NCORES = 8
GT, GH, GW = 8, 16, 16
SCALE = 1.0 / math.sqrt(HD)

NQ = 4            # token quarters
QS = N // NQ      # 512
KT = N // 128     # 16 k-tiles
CK = C // 128     # 9 contraction chunks of x
# e_t inventory: phase-1 units get dedicated slots (E1 tiles, no recycling,
# so emission order is unconstrained); phase-2 units recycle E2 slots in
# strict consumption order.
E1 = 40
E2 = 16

# phase-1 QK row tiles: T0..T3 are 128 rows, T4 is 64.
# Within a 96-row block rows are j-major: row = 4*j + h.
#  T0 = [Q_E(96); K_E j0..7]   T1 = [Q_O(96); K_O j0..7]
#  T2 = [K_E j8..23; Q_P j0..15]
#  T3 = [K_O j8..23; Q_P j16..23; K_P j0..7]
#  T4 = [K_P j8..23]
QK_ROWS = 576


def _axis_freqs(n: int) -> np.ndarray:
    base = np.linspace(1.0, 128.0, 8, dtype=np.float64) * np.pi  # MAX_FREQ/2
    pos = np.linspace(-1.0, 1.0, n, dtype=np.float64)
    return pos[:, None] * base[None, :]


def _freq24() -> np.ndarray:
    """per-token frequency of rotary pair j (24 pairs) -> (N, 24)."""
    f = np.zeros((GT, GH, GW, 24), dtype=np.float64)
    f[..., 0:8] = _axis_freqs(GT)[:, None, None, :]
    f[..., 8:16] = _axis_freqs(GH)[None, :, None, :]
    f[..., 16:24] = _axis_freqs(GW)[None, None, :, :]
    return f.reshape(N, 24)


def _host_tables():
    f24 = _freq24()                      # (N, 24)
    cos24 = np.cos(f24).T                # (24, N) float64
    sin24 = np.sin(f24).T
    # cs0: rows r<96: j=r//4 ; rows 96..127: j=(r-96)//4
    j0 = np.concatenate([np.arange(96) // 4, np.arange(32) // 4])
    # cs2: rows 0..63: j = 8 + r//4
    j2 = 8 + np.arange(64) // 4
    bf = ml_dtypes.bfloat16
    return (
        cos24[j0].astype(bf), sin24[j0].astype(bf),
        cos24[j2].astype(bf), sin24[j2].astype(bf),
    )


def build_nc() -> bass.Bass:
    nc = bass.Bass()
    xT = nc.dram_tensor("xT", [C, N], BF16, kind="ExternalInput")
    # wqkv = [wqk rows (576) ; wv cols (288)] fused per contraction chunk
    wqkv = nc.dram_tensor("wqkv", [C, QK_ROWS + HPG * HD], BF16,
                          kind="ExternalInput")
    wpAB = nc.dram_tensor("wpAB", [128, 2, C], BF16, kind="ExternalInput")
    wpC = nc.dram_tensor("wpC", [33, C], BF16, kind="ExternalInput")
    cs0 = nc.dram_tensor("cs0", [128, 2, N], BF16, kind="ExternalInput")
    cs2 = nc.dram_tensor("cs2", [64, 2, N], BF16, kind="ExternalInput")
    iden = nc.dram_tensor("iden", [128, 128], BF16, kind="ExternalInput")
    outT = nc.dram_tensor("outT", [C, N], F32, kind="ExternalOutput")

    with tile.TileContext(nc) as tc:
        with tc.tile_pool(name="persist", bufs=1) as pp:
            # fp8 q/k in DoubleRow block layout: [36, head, slot, token]
            qil = pp.tile([36, HPG, 2, N], F8, name="qil")
            kil = pp.tile([36, HPG, 2, N], F8, name="kil")
            # v with ones column: [token128, ktile, head, 80] (73 used)
            v_all = pp.tile([128, KT, HPG, 80], BF16, name="v_all")
            # packed o^T rows for the projection: 289 rows in 128+128+33
            otA = pp.tile([128, N], BF16, name="otA")
            otB = pp.tile([128, N], BF16, name="otB")
            otC = pp.tile([33, N], BF16, name="otC")
            iden_t = pp.tile([128, 128], BF16, name="iden_t")
            cs0_t = pp.tile([128, 2, N], BF16, name="cs0_t")
            cs2_t = pp.tile([64, 2, N], BF16, name="cs2_t")
            wpAB_t = pp.tile([128, 2, C], BF16, name="wpAB_t")
            wpC_t = pp.tile([33, C], BF16, name="wpC_t")
            cs0c_t, cs0s_t = cs0_t[:, 0], cs0_t[:, 1]
            cs2c_t, cs2s_t = cs2_t[:, 0], cs2_t[:, 1]
            wp_t = {"A": wpAB_t[:, 0], "B": wpAB_t[:, 1], "C": wpC_t[:]}
            nc.vector.memset(v_all[:, :, :, HD], 1.0)
            nc.vector.memset(otC[32:33, :], 1.0)

            # ---- exp units -----------------------------------------------
            # unit = (jq, h, kp): scores for q-chunk jq (512 q) against
            # k-tiles 2kp, 2kp+1 -> one exp of [128, 1024] -> e_t (bf16).
            e_tiles = {}
            emitted = set()

            def emit_unit(jq, h, kp, ps_pool, st_tag, st_bufs, e_tag, e_bufs,
                          e_dt):
                st = ps_pool.tile([128, 2 * QS], F32, tag=st_tag,
                                  bufs=st_bufs, name=f"st_{jq}_{h}_{kp}")
                for i in range(2):
                    kt = 2 * kp + i
                    nc.tensor.matmul(
                        st[:, i * QS:(i + 1) * QS],
                        kil[:, h, :, kt * 128:(kt + 1) * 128],
                        qil[:, h, :, jq * QS:(jq + 1) * QS],
                        start=True, stop=True, perf_mode=DR,
                    )
                e_t = pp.tile([128, 2 * QS], e_dt, tag=e_tag, bufs=e_bufs,
                              name=f"e_{jq}_{h}_{kp}")
                nc.scalar.activation(e_t[:], st[:], AF.Exp, scale=SCALE)
                e_tiles[(jq, h, kp)] = e_t
                emitted.add((jq, h, kp))

            consume_order = [
                (jq, h, kp) for jq in range(NQ) for h in range(HPG)
                for kp in range(KT // 2)
            ]
            # phase-1 units: spread across jq<=2 blocks (kp<=2 each, plus
            # kp=3 for jq0) so every phase-2 block retains >=4 live exp units
            # to hide its PV->norm->drain latency chain behind ACT work
            p1_units = [u for u in consume_order
                        if (u[0] <= 2 and u[2] <= 2)
                        or (u[0] == 0 and u[2] == 3)][:E1]
            avail = []

            def drip(n):
                for _ in range(n):
                    if not avail:
                        return
                    jq, h, kp = avail.pop(0)
                    emit_unit(jq, h, kp, ps1_ref[0], "st1", 2, "e1", E1, F8)

            # ================= phase 1: QKV + RoPE + repack ================
            with (
                tc.tile_pool(name="p1", bufs=1) as p1,
                tc.tile_pool(name="psum1", bufs=1, space="PSUM") as ps1,
            ):
                ps1_ref = [ps1]
                wqkv_t = [p1.tile([128, QK_ROWS + HPG * HD], BF16,
                                  name=f"wqkv{k}") for k in range(CK)]
                wqk_t = [t[:, 0:QK_ROWS] for t in wqkv_t]
                wv_t = [t[:, QK_ROWS:] for t in wqkv_t]

                ro = {}  # rope-output tiles, allocated per token-half

                def emit_v(vq, vxq):
                    for tt in range(4):
                        v_ps = ps1.tile([128, QS], F32, tag="qk",
                                        bufs=4, name=f"vps{vq}_{tt}")
                        for k in range(CK):
                            nc.tensor.matmul(
                                v_ps[:, 0:HPG * HD],
                                vxq[k][:, tt * 128:(tt + 1) * 128],
                                wv_t[k][:],
                                start=(k == 0), stop=(k == CK - 1),
                            )
                            if k % 4 == 1:
                                drip(1)
                        nc.vector.tensor_copy(
                            v_all[:, vq * 4 + tt, :, 0:HD],
                            v_ps[:, 0:HPG * HD]
                            .rearrange("p (h d) -> p h d", h=HPG),
                        )

                for qn in range(NQ):
                    ts0 = qn * QS
                    hs0 = (qn // 2) * 2 * QS   # token base of this half
                    qo = (qn % 2) * QS         # quarter offset within half
                    if qn % 2 == 0:
                        # x loads at half-token granularity: half the DMA
                        # count on the serial DGE front-end, and the odd
                        # quarters never wait on x at all
                        xh = [p1.tile([128, 2 * QS], BF16, tag=f"xq{k}",
                                      bufs=1, name=f"xq{k}_{qn}")
                              for k in range(CK)]
                    xq = [t[:, qo:qo + QS] for t in xh]
                    for k in range(CK):
                        if qn % 2 == 0:
                            nc.sync.dma_start(
                                xh[k][:],
                                xT[k * 128:(k + 1) * 128, hs0:hs0 + 2 * QS])
                        if qn == 0:
                            nc.sync.dma_start(
                                wqkv_t[k][:], wqkv[k * 128:(k + 1) * 128, :])
                            if k == 1:
                                # cs tables ride behind the first two chunk
                                # pairs so the PE can start immediately
                                nc.sync.dma_start(cs0_t[:], cs0.ap())
                                nc.sync.dma_start(cs2_t[:], cs2.ap())
                    if qn == 0:
                        nc.sync.dma_start(wpAB_t[:], wpAB.ap())
                        nc.sync.dma_start(wpC_t[:], wpC.ap())
                        nc.sync.dma_start(iden_t[:], iden.ap())

                    qk_ps = {}

                    def qk_wave(tiles, rows=128, qn=qn, xq=xq, qk_ps=qk_ps):
                        for t in tiles:
                            qk_ps[t] = ps1.tile(
                                [128, QS], F32, tag="qk", bufs=4,
                                name=f"qk{t}_{qn}")
                        for k in range(CK):
                            for t in tiles:
                                nc.tensor.matmul(
                                    qk_ps[t][0:rows, :],
                                    wqk_t[k][:, t * 128:t * 128 + rows],
                                    xq[k][:],
                                    start=(k == 0), stop=(k == CK - 1),
                                )
                            if k % 2 == 1:
                                # drip sites spaced ~1us: matches ACT pace
                                drip(1)

                    def rope_pair(e_ps, o_ps, rows, cosA, sinA, er, orr,
                                  qn=qn, ts0=ts0, qo=qo):
                        """er = e*cos - o*sin ; or = o*cos + e*sin (fp8 out)"""
                        sl = slice(0, rows)
                        cs = cosA[sl, ts0:ts0 + QS]
                        sn = sinA[sl, ts0:ts0 + QS]
                        osl = slice(qo, qo + QS)
                        t1 = p1.tile([128, QS], F32, tag="rtA", bufs=2,
                                     name=f"t1_{qn}_{rows}")
                        t2 = p1.tile([128, QS], F32, tag="rtB", bufs=2,
                                     name=f"t2_{qn}_{rows}")
                        nc.vector.tensor_tensor(t1[sl], e_ps[sl], cs, ALU.mult)
                        nc.vector.tensor_tensor(t2[sl], o_ps[sl], sn, ALU.mult)
                        nc.vector.tensor_tensor(er[sl, osl], t1[sl], t2[sl],
                                                ALU.subtract)
                        t3 = p1.tile([128, QS], F32, tag="rtA", bufs=2,
                                     name=f"t3_{qn}_{rows}")
                        t4 = p1.tile([128, QS], F32, tag="rtB", bufs=2,
                                     name=f"t4_{qn}_{rows}")
                        nc.vector.tensor_tensor(t3[sl], o_ps[sl], cs, ALU.mult)
                        nc.vector.tensor_tensor(t4[sl], e_ps[sl], sn, ALU.mult)
                        nc.vector.tensor_tensor(orr[sl, osl], t3[sl], t4[sl],
                                                ALU.add)

                    if qn % 2 == 0:
                        # rope-output tiles span a token half (2 quarters)
                        ro = {
                            nm: p1.tile([rows, 2 * QS], F8, tag=nm, bufs=2,
                                        name=f"{nm}_{qn // 2}")
                            for nm, rows in (
                                ("er0", 128), ("or0", 128), ("er2", 64),
                                ("or2", 64), ("pq", 128), ("pq2", 128),
                                ("pk1", 128), ("pk2", 64),
                            )
                        }

                    def repack(dst_il, p0, src, s0, nj, slot, c0, cw,
                               hs0=hs0):
                        """src rows s0..s0+4nj (j-major, 4 heads), cols
                        c0..c0+cw -> dst_il partitions p0..p0+nj-1."""
                        nc.sync.dma_start(
                            dst_il[p0:p0 + nj, :, slot, hs0 + c0:hs0 + c0 + cw],
                            src[s0:s0 + 4 * nj, c0:c0 + cw],
                        )

                    # wave 1: T0, T1 -> rope full pair
                    qk_wave([0, 1])
                    rope_pair(qk_ps[0], qk_ps[1], 128, cs0c_t, cs0s_t,
                              ro["er0"], ro["or0"])

                    # wave 2: T2, T3 -> pass copies first (so the k-side
                    # repack isn't stuck behind both rope waves), then rope
                    qk_wave([2, 3])
                    osl = slice(qo, qo + QS)
                    # quarter 0: ACT is idle until this repack completes, so
                    # the pass copies ride there, shortening the DVE chain
                    ceng = nc.scalar.copy if qn == 0 else nc.vector.tensor_copy
                    ceng(ro["pq"][64:128, osl], qk_ps[2][64:128, :])
                    ceng(ro["pq2"][64:96, osl], qk_ps[3][64:96, :])
                    ceng(ro["pk1"][96:128, osl], qk_ps[3][96:128, :])

                    # wave 3: T4 (QK rows only; V is deferred so this
                    # quarter's repack completes as early as possible)
                    qk_wave([4], rows=64)
                    ceng(ro["pk2"][0:64, osl], qk_ps[4][0:64, :])
                    rope_pair(qk_ps[2], qk_ps[3], 64, cs2c_t, cs2s_t,
                              ro["er2"], ro["or2"])

                    # quarter-wise repack into the fp8 DoubleRow layout
                    c0, cw = qo, QS
                    repack(qil, 0, ro["er0"], 0, 24, 0, c0, cw)
                    repack(kil, 0, ro["er0"], 96, 8, 0, c0, cw)
                    repack(qil, 24, ro["or0"], 0, 12, 0, c0, cw)
                    repack(qil, 0, ro["or0"], 48, 12, 1, c0, cw)
                    repack(kil, 24, ro["or0"], 96, 8, 0, c0, cw)
                    repack(kil, 8, ro["er2"], 0, 16, 0, c0, cw)
                    repack(kil, 32, ro["or2"], 0, 4, 0, c0, cw)
                    repack(kil, 0, ro["or2"], 16, 12, 1, c0, cw)
                    repack(qil, 12, ro["pq"], 64, 16, 1, c0, cw)
                    repack(qil, 28, ro["pq2"], 64, 8, 1, c0, cw)
                    repack(kil, 12, ro["pk1"], 96, 8, 1, c0, cw)
                    repack(kil, 20, ro["pk2"], 0, 16, 1, c0, cw)

                    # units made available by this quarter's repack, in
                    # consume order (phase-1 slots are never recycled, so
                    # availability order cannot deadlock the slot ring)
                    for u in p1_units:
                        if u not in emitted and u not in avail:
                            if u[0] <= qn and u[2] // 2 <= qn:
                                avail.append(u)
                    drip(3)
                    emit_v(qn, xq)

                # drain leftover phase-1 units, then finish block (0,0)'s
                # two remaining score units so PV can start at phase-2 entry
                drip(len(avail))
                for kp in range(KT // 2):
                    if (0, 0, kp) not in emitted:
                        emit_unit(0, 0, kp, ps1, "st1", 2, "e2", E2, BF16)

            # ================= phase 2: attention + projection =============
            with (
                tc.tile_pool(name="p2", bufs=1) as p2,
                tc.tile_pool(name="psum2", bufs=1, space="PSUM") as ps2,
            ):
                pending = []
                o_sb_cur = [None]
                parked = {}

                def emit_proj(jq, ct, mode="full"):
                    # mode "A": chunk A only, parked to SBUF (bf16)
                    # mode "B": chunks B+C plus the parked partial, store
                    sl = slice(jq * QS, (jq + 1) * QS)
                    cs = slice(ct * 128, (ct + 1) * 128)
                    o_ps = ps2.tile([128, QS], F32, tag="o_ps", bufs=1,
                                    name=f"ops{jq}_{ct}_{mode}")
                    if mode != "B":
                        nc.tensor.matmul(o_ps[:], wp_t["A"][:, cs], otA[:, sl],
                                         start=True, stop=(mode == "A"))
                    if mode != "A":
                        nc.tensor.matmul(o_ps[:], wp_t["B"][:, cs], otB[:, sl],
                                         start=(mode == "B"), stop=False)
                        nc.tensor.matmul(o_ps[:], wp_t["C"][:, cs], otC[:, sl],
                                         start=False, stop=(mode != "B"))
                    if mode == "B":
                        # fold the parked chunk-A partial back in on the PE
                        nc.tensor.matmul(o_ps[:], iden_t[:], parked[ct][:],
                                         start=False, stop=True)
                    if mode == "A":
                        prk = p2.tile([128, QS], BF16, tag=f"prk{ct}", bufs=1,
                                      name=f"prk{ct}")
                        nc.scalar.copy(prk[:], o_ps[:])
                        parked[ct] = prk
                        return
                    half = ct % 2
                    if half == 0:
                        o_sb_cur[0] = p2.tile([128, 2, QS], F32, tag="o_sb",
                                              bufs=3, name=f"osb{jq}_{ct}")
                    o_sb = o_sb_cur[0]
                    if mode == "B":
                        # ACT is idle after the last exp; drain there
                        nc.scalar.copy(o_sb[:, half, :], o_ps[:])
                    else:
                        nc.vector.tensor_copy(o_sb[:, half, :], o_ps[:])
                    if half == 1 or ct == CK - 1:
                        for i in range(half + 1):
                            r0 = (ct - half + i) * 128
                            nc.sync.dma_start(outT[r0:r0 + 128, sl],
                                              o_sb[:, i, :])

                blocks = [(jq, h) for jq in range(NQ) for h in range(HPG)]

                def emit_units_for(blk):
                    jq, h = blk
                    for kp in range(KT // 2):
                        if (jq, h, kp) not in emitted:
                            emit_unit(jq, h, kp, ps2, "st2", 2, "e2", E2,
                                      BF16)

                emit_units_for(blocks[0])
                for bi, (jq, h) in enumerate(blocks):
                    # software pipeline: exps for the next block first
                    if bi + 1 < len(blocks):
                        emit_units_for(blocks[bi + 1])
                    # PV: out [128 q, 4, 73]; stationary = e chunks
                    pv = ps2.tile([128, 4, 73], F32, tag="pv", bufs=1,
                                  name=f"pv{jq}_{h}")
                    # the PSUM start bit zeroes a whole 2KB bank, which would
                    # wipe sibling q-tile accumulators sharing this bank; so
                    # pre-zero once and accumulate with start=False
                    nc.vector.memset(pv[:], 0.0)
                    # kt-outer so each e_t tile's last read comes early and
                    # its slot frees progressively
                    for kt in range(KT):
                        e_t = e_tiles[(jq, h, kt // 2)]
                        for qt in range(4):
                            c0 = (kt % 2) * QS + qt * 128
                            nc.tensor.matmul(
                                pv[:, qt, :],
                                e_t[:, c0:c0 + 128],
                                v_all[:, kt, h, 0:HD + 1],
                                start=False, stop=(kt == KT - 1),
                                skip_group_check=True,
                            )
                    rec = p2.tile([128, 4], F32, tag="rec", bufs=2,
                                  name=f"rec{jq}_{h}")
                    nc.vector.reciprocal(rec[:], pv[:, :, HD])
                    # each transpose gets a fresh psum tile (its start
                    # bit zeroes the whole bank) and is drained immediately
                    tr_sb = p2.tile([HD, QS], BF16, tag="tr_sb", bufs=2,
                                    name=f"trsb{jq}_{h}")
                    for qt in range(4):
                        onr = p2.tile([128, HD], BF16, tag="onr", bufs=3,
                                      name=f"onr{jq}_{h}_{qt}")
                        nc.vector.tensor_scalar_mul(
                            onr[:], pv[:, qt, 0:HD], rec[:, qt:qt + 1])
                        tr = ps2.tile([HD, 128], BF16, tag="tr", bufs=2,
                                      name=f"tr{jq}_{h}_{qt}")
                        nc.tensor.transpose(tr[:], onr[:], iden_t[:])
                        nc.vector.tensor_copy(
                            tr_sb[:, qt * 128:(qt + 1) * 128], tr[:])
                    sl = slice(jq * QS, (jq + 1) * QS)
                    r0 = h * HD
                    for dst, lo, hi in (
                        (otA, 0, 128), (otB, 128, 256), (otC, 256, 288),
                    ):
                        a, b_ = max(lo, r0), min(hi, r0 + HD)
                        if a < b_:
                            nc.sync.dma_start(
                                dst[a - lo:b_ - lo, sl],
                                tr_sb[a - r0:b_ - r0, :])
                    for _ in range(6):
                        if pending:
                            emit_proj(*pending.pop(0))
                    if h == HPG - 1 and jq < NQ - 1:
                        pending.extend((jq, ct) for ct in range(CK))
                    if jq == NQ - 1 and h == 1:
                        # last-q-chunk tail split: chunk A (heads 0-1 rows)
                        # can project as soon as their o^T rows land
                        pending.extend((NQ - 1, ct, "A") for ct in range(CK))
                while pending:
                    emit_proj(*pending.pop(0))
                for ct in range(CK):
                    emit_proj(NQ - 1, ct, "B")

    bass_rust.generate_event_semaphores(nc)
    return nc


_NC = None


def _get_nc():
    global _NC
    if _NC is None:
        _NC = build_nc()
    return _NC


def _pack_inputs(x, Wqkv, Wproj, bproj):
    bf = ml_dtypes.bfloat16
    cos0, sin0, cos2, sin2 = _host_tables()
    iden = np.eye(128, dtype=bf)

    def qcol(g, h, d):  # q column of head 4g+h dim d
        return (4 * g + h) * HD + d

    def kcol(g, h, d):
        return C + (4 * g + h) * HD + d

    in_maps = []
    for core in range(NCORES):
        b, g = divmod(core, HPG)
        cols = []
        for j in range(24):          # T0: Q_E ; K_E j0..7
            cols += [qcol(g, h, 2 * j) for h in range(4)]
        for j in range(8):
            cols += [kcol(g, h, 2 * j) for h in range(4)]
        for j in range(24):          # T1: Q_O ; K_O j0..7
            cols += [qcol(g, h, 2 * j + 1) for h in range(4)]
        for j in range(8):
            cols += [kcol(g, h, 2 * j + 1) for h in range(4)]
        for j in range(8, 24):       # T2: K_E j8..23 ; Q_P j0..15
            cols += [kcol(g, h, 2 * j) for h in range(4)]
        for j in range(16):
            cols += [qcol(g, h, 48 + j) for h in range(4)]
        for j in range(8, 24):       # T3: K_O j8..23 ; Q_P j16..23 ; K_P j0..7
            cols += [kcol(g, h, 2 * j + 1) for h in range(4)]
        for j in range(16, 24):
            cols += [qcol(g, h, 48 + j) for h in range(4)]
        for j in range(8):
            cols += [kcol(g, h, 48 + j) for h in range(4)]
        for j in range(8, 24):       # T4: K_P j8..23
            cols += [kcol(g, h, 48 + j) for h in range(4)]
        assert len(cols) == QK_ROWS

        cols += list(range(2 * C + 4 * g * HD, 2 * C + (4 * g + 4) * HD))
        wqkv_c = Wqkv[:, cols].astype(bf)                           # [C, 864]
        wp_rows = Wproj[4 * g * HD:(4 * g + 4) * HD, :].astype(bf)  # [288, C]
        wpAB = np.stack([wp_rows[0:128], wp_rows[128:256]], axis=1)
        wpC = np.zeros((33, C), dtype=bf)
        wpC[0:32] = wp_rows[256:288]
        wpC[32] = (bproj if g == 0 else np.zeros_like(bproj)).astype(bf)
        cs0 = np.stack([cos0, sin0], axis=1)    # [128, 2, N]
        cs2 = np.stack([cos2, sin2], axis=1)    # [64, 2, N]

        in_maps.append({
            "xT": np.ascontiguousarray(x[b].T).astype(bf).view(np.uint16),
            "wqkv": np.ascontiguousarray(wqkv_c).view(np.uint16),
            "wpAB": np.ascontiguousarray(wpAB).view(np.uint16),
            "wpC": wpC.view(np.uint16),
            "cs0": np.ascontiguousarray(cs0).view(np.uint16),
            "cs2": np.ascontiguousarray(cs2).view(np.uint16),
            "iden": iden.view(np.uint16),
        })
    return in_maps


def kernel(x, Wqkv, Wproj, bproj, T, H, W):
    x = np.asarray(x, dtype=np.float32)
    Wqkv = np.asarray(Wqkv, dtype=np.float32)
    Wproj = np.asarray(Wproj, dtype=np.float32)
    bproj = np.asarray(bproj, dtype=np.float32)
    assert x.shape == (B, N, C) and Wqkv.shape == (C, 3 * C)
    assert (int(T), int(H), int(W)) == (GT, GH, GW)

    nc = _get_nc()
    in_maps = _pack_inputs(x, Wqkv, Wproj, bproj)
    res = run_bass_kernel_spmd(nc, in_maps, core_ids=list(range(NCORES)))
    out = np.zeros((B, N, C), dtype=np.float32)
    for core in range(NCORES):
        r = np.asarray(res.results[core]["outT"])
        if r.dtype != np.float32:
            r = r.view(ml_dtypes.bfloat16).astype(np.float32)
        out[core // HPG] += r.T
    return out


# revision 71
# speedup vs baseline: 1.0108x; 1.0108x over previous
"""Trainium2 Bass kernel for nn_Attention_79224966742132.

Dense transformer attention block: QKV projection + axial RoPE + SDPA +
output projection, for x (2, 2048, 1152), 16 heads of dim 72.

Sharding (8 cores): data-parallel over batch (2) x tensor-parallel over
head groups (4 heads/core). Each core computes QKV for its 4 heads from
the full x[b], applies RoPE, runs attention, and produces a partial
output projection (row-parallel Wproj); the host sums the 4 partials per
batch element. The projection bias rides on the g==0 core of each batch.

Engine plan (per the TRN2 cost model):
 - all GEMM inputs bf16 (1 cycle/row); scores q/k quantized to fp8e4 and
   run in DoubleRow perf mode (0.5 cycles/row) with the 72-dim
   contraction packed as [36 partitions x 2 slots].
 - exp of all N^2 scores is the critical ~130us ACT chain; score->exp
   units are interleaved into phase 1 (capped by the e_t inventory E so
   the in-order PE queue can never block on a PV-gated slot) so ACT
   starts early and never runs dry afterwards.
 - PV is "flipped": out [128 q, 73] with the exp tile as stationary, so
   each matmul streams only F=73 rows; a ones column in V yields the
   softmax denominator; normalize via per-partition tensor_scalar; PE
   transposes (bf16) restore o^T for the row-parallel projection.
 - projection contraction packed to [128,128,33] chunks with the bias as
   a ones-row in the last chunk (o^T rows staged via aligned DVE copy +
   partition-arbitrary SBUF->SBUF DMA).
"""
import math
import os
import sys

# The device path needs the axon/neuron jax platform; if a harness pinned
# JAX_PLATFORMS=cpu (common for running jax references) and jax is not yet
# imported, restore platform auto-detection.
if "jax" not in sys.modules:
    _jp = os.environ.get("JAX_PLATFORMS")
    if _jp and "axon" not in _jp and "neuron" not in _jp:
        del os.environ["JAX_PLATFORMS"]

import numpy as np
import ml_dtypes

import bass_rust
import concourse.bass as bass
import concourse.mybir as mybir
import concourse.tile as tile
from concourse.bass_utils import run_bass_kernel_spmd

F32 = mybir.dt.float32
BF16 = mybir.dt.bfloat16
F8 = mybir.dt.float8e4
AF = mybir.ActivationFunctionType
ALU = mybir.AluOpType
DR = mybir.MatmulPerfMode.DoubleRow

B = 2
N = 2048          # tokens = T*H*W = 8*16*16
C = 1152
NH = 16
HD = 72
HPG = 4           # heads per core
NCORES = 8
GT, GH, GW = 8, 16, 16
SCALE = 1.0 / math.sqrt(HD)

NQ = 4            # token quarters
QS = N // NQ      # 512
KT = N // 128     # 16 k-tiles
CK = C // 128     # 9 contraction chunks of x
# e_t inventory: phase-1 units get dedicated slots (E1 tiles, no recycling,
# so emission order is unconstrained); phase-2 units recycle E2 slots in
# strict consumption order.
E1 = 40
E2 = 16

# phase-1 QK row tiles: T0..T3 are 128 rows, T4 is 64.
# Within a 96-row block rows are j-major: row = 4*j + h.
#  T0 = [Q_E(96); K_E j0..7]   T1 = [Q_O(96); K_O j0..7]
#  T2 = [K_E j8..23; Q_P j0..15]
#  T3 = [K_O j8..23; Q_P j16..23; K_P j0..7]
#  T4 = [K_P j8..23]
QK_ROWS = 576


def _axis_freqs(n: int) -> np.ndarray:
    base = np.linspace(1.0, 128.0, 8, dtype=np.float64) * np.pi  # MAX_FREQ/2
    pos = np.linspace(-1.0, 1.0, n, dtype=np.float64)
    return pos[:, None] * base[None, :]


def _freq24() -> np.ndarray:
    """per-token frequency of rotary pair j (24 pairs) -> (N, 24)."""
    f = np.zeros((GT, GH, GW, 24), dtype=np.float64)
    f[..., 0:8] = _axis_freqs(GT)[:, None, None, :]
    f[..., 8:16] = _axis_freqs(GH)[None, :, None, :]
    f[..., 16:24] = _axis_freqs(GW)[None, None, :, :]
    return f.reshape(N, 24)


def _host_tables():
    f24 = _freq24()                      # (N, 24)
    cos24 = np.cos(f24).T                # (24, N) float64
    sin24 = np.sin(f24).T
    # cs0: rows r<96: j=r//4 ; rows 96..127: j=(r-96)//4
    j0 = np.concatenate([np.arange(96) // 4, np.arange(32) // 4])
    # cs2: rows 0..63: j = 8 + r//4
    j2 = 8 + np.arange(64) // 4
    bf = ml_dtypes.bfloat16
    return (
        cos24[j0].astype(bf), sin24[j0].astype(bf),
        cos24[j2].astype(bf), sin24[j2].astype(bf),
    )


def build_nc() -> bass.Bass:
    nc = bass.Bass()
    xT = nc.dram_tensor("xT", [C, N], BF16, kind="ExternalInput")
    # wqkv = [wqk rows (576) ; wv cols (288)] fused per contraction chunk
    wqkv = nc.dram_tensor("wqkv", [C, QK_ROWS + HPG * HD], BF16,
                          kind="ExternalInput")
    wpAB = nc.dram_tensor("wpAB", [128, 2, C], BF16, kind="ExternalInput")
    wpC = nc.dram_tensor("wpC", [33, C], BF16, kind="ExternalInput")
    cs0 = nc.dram_tensor("cs0", [128, 2, N], BF16, kind="ExternalInput")
    cs2 = nc.dram_tensor("cs2", [64, 2, N], BF16, kind="ExternalInput")
    iden = nc.dram_tensor("iden", [128, 128], BF16, kind="ExternalInput")
    outT = nc.dram_tensor("outT", [C, N], F32, kind="ExternalOutput")

    with tile.TileContext(nc) as tc:
        with tc.tile_pool(name="persist", bufs=1) as pp:
            # fp8 q/k in DoubleRow block layout: [36, head, slot, token]
            qil = pp.tile([36, HPG, 2, N], F8, name="qil")
            kil = pp.tile([36, HPG, 2, N], F8, name="kil")
            # v with ones column: [token128, ktile, head, 80] (73 used)
            v_all = pp.tile([128, KT, HPG, 80], BF16, name="v_all")
            # packed o^T rows for the projection: 289 rows in 128+128+33
            otA = pp.tile([128, N], BF16, name="otA")
            otB = pp.tile([128, N], BF16, name="otB")
            otC = pp.tile([33, N], BF16, name="otC")
            iden_t = pp.tile([128, 128], BF16, name="iden_t")
            cs0_t = pp.tile([128, 2, N], BF16, name="cs0_t")
            cs2_t = pp.tile([64, 2, N], BF16, name="cs2_t")
            wpAB_t = pp.tile([128, 2, C], BF16, name="wpAB_t")
            wpC_t = pp.tile([33, C], BF16, name="wpC_t")
            cs0c_t, cs0s_t = cs0_t[:, 0], cs0_t[:, 1]
            cs2c_t, cs2s_t = cs2_t[:, 0], cs2_t[:, 1]
            wp_t = {"A": wpAB_t[:, 0], "B": wpAB_t[:, 1], "C": wpC_t[:]}
            nc.vector.memset(v_all[:, :, :, HD], 1.0)
            nc.vector.memset(otC[32:33, :], 1.0)

            # ---- exp units -----------------------------------------------
            # unit = (jq, h, kp): scores for q-chunk jq (512 q) against
            # k-tiles 2kp, 2kp+1 -> one exp of [128, 1024] -> e_t (bf16).
            e_tiles = {}
            emitted = set()

            def emit_unit(jq, h, kp, ps_pool, st_tag, st_bufs, e_tag, e_bufs,
                          e_dt):
                st = ps_pool.tile([128, 2 * QS], F32, tag=st_tag,
                                  bufs=st_bufs, name=f"st_{jq}_{h}_{kp}")
                for i in range(2):
                    kt = 2 * kp + i
                    nc.tensor.matmul(
                        st[:, i * QS:(i + 1) * QS],
                        kil[:, h, :, kt * 128:(kt + 1) * 128],
                        qil[:, h, :, jq * QS:(jq + 1) * QS],
                        start=True, stop=True, perf_mode=DR,
                    )
                e_t = pp.tile([128, 2 * QS], e_dt, tag=e_tag, bufs=e_bufs,
                              name=f"e_{jq}_{h}_{kp}")
                nc.scalar.activation(e_t[:], st[:], AF.Exp, scale=SCALE)
                e_tiles[(jq, h, kp)] = e_t
                emitted.add((jq, h, kp))

            consume_order = [
                (jq, h, kp) for jq in range(NQ) for h in range(HPG)
                for kp in range(KT // 2)
            ]
            # phase-1 units: spread across jq<=2 blocks (kp<=2 each, plus
            # kp=3 for jq0) so every phase-2 block retains >=4 live exp units
            # to hide its PV->norm->drain latency chain behind ACT work
            p1_units = [u for u in consume_order
                        if (u[0] <= 2 and u[2] <= 2)
                        or (u[0] == 0 and u[2] == 3)][:E1]
            avail = []

            def drip(n):
                for _ in range(n):
                    if not avail:
                        return
                    jq, h, kp = avail.pop(0)
                    emit_unit(jq, h, kp, ps1_ref[0], "st1", 2, "e1", E1, F8)

            # ================= phase 1: QKV + RoPE + repack ================
            with (
                tc.tile_pool(name="p1", bufs=1) as p1,
                tc.tile_pool(name="psum1", bufs=1, space="PSUM") as ps1,
            ):
                ps1_ref = [ps1]
                wqkv_t = [p1.tile([128, QK_ROWS + HPG * HD], BF16,
                                  name=f"wqkv{k}") for k in range(CK)]
                wqk_t = [t[:, 0:QK_ROWS] for t in wqkv_t]
                wv_t = [t[:, QK_ROWS:] for t in wqkv_t]

                ro = {}  # rope-output tiles, allocated per token-half

                def emit_v(vq, vxq):
                    for tt in range(4):
                        v_ps = ps1.tile([128, QS], F32, tag="qk",
                                        bufs=4, name=f"vps{vq}_{tt}")
                        for k in range(CK):
                            nc.tensor.matmul(
                                v_ps[:, 0:HPG * HD],
                                vxq[k][:, tt * 128:(tt + 1) * 128],
                                wv_t[k][:],
                                start=(k == 0), stop=(k == CK - 1),
                            )
                            if k % 4 == 1:
                                drip(1)
                        nc.vector.tensor_copy(
                            v_all[:, vq * 4 + tt, :, 0:HD],
                            v_ps[:, 0:HPG * HD]
                            .rearrange("p (h d) -> p h d", h=HPG),
                        )

                for qn in range(NQ):
                    ts0 = qn * QS
                    hs0 = (qn // 2) * 2 * QS   # token base of this half
                    qo = (qn % 2) * QS         # quarter offset within half
                    if qn % 2 == 0:
                        # x loads at half-token granularity: half the DMA
                        # count on the serial DGE front-end, and the odd
                        # quarters never wait on x at all
                        xh = [p1.tile([128, 2 * QS], BF16, tag=f"xq{k}",
                                      bufs=1, name=f"xq{k}_{qn}")
                              for k in range(CK)]
                    xq = [t[:, qo:qo + QS] for t in xh]
                    for k in range(CK):
                        if qn % 2 == 0:
                            nc.sync.dma_start(
                                xh[k][:],
                                xT[k * 128:(k + 1) * 128, hs0:hs0 + 2 * QS])
                        if qn == 0:
                            nc.sync.dma_start(
                                wqkv_t[k][:], wqkv[k * 128:(k + 1) * 128, :])
                            if k == 1:
                                # cs tables ride behind the first two chunk
                                # pairs so the PE can start immediately
                                nc.sync.dma_start(cs0_t[:], cs0.ap())
                                nc.sync.dma_start(cs2_t[:], cs2.ap())
                    if qn == 0:
                        nc.sync.dma_start(wpAB_t[:], wpAB.ap())
                        nc.sync.dma_start(wpC_t[:], wpC.ap())
                        nc.sync.dma_start(iden_t[:], iden.ap())

                    qk_ps = {}

                    def qk_wave(tiles, rows=128, qn=qn, xq=xq, qk_ps=qk_ps):
                        for t in tiles:
                            qk_ps[t] = ps1.tile(
                                [128, QS], F32, tag="qk", bufs=4,
                                name=f"qk{t}_{qn}")
                        for k in range(CK):
                            for t in tiles:
                                nc.tensor.matmul(
                                    qk_ps[t][0:rows, :],
                                    wqk_t[k][:, t * 128:t * 128 + rows],
                                    xq[k][:],
                                    start=(k == 0), stop=(k == CK - 1),
                                )
                            if k % 2 == 1:
                                # drip sites spaced ~1us: matches ACT pace
                                drip(1)

                    def rope_pair(e_ps, o_ps, rows, cosA, sinA, er, orr,
                                  qn=qn, ts0=ts0, qo=qo):
                        """er = e*cos - o*sin ; or = o*cos + e*sin (fp8 out)"""
                        sl = slice(0, rows)
                        cs = cosA[sl, ts0:ts0 + QS]
                        sn = sinA[sl, ts0:ts0 + QS]
                        osl = slice(qo, qo + QS)
                        t1 = p1.tile([128, QS], F32, tag="rtA", bufs=2,
                                     name=f"t1_{qn}_{rows}")
                        t2 = p1.tile([128, QS], F32, tag="rtB", bufs=2,
                                     name=f"t2_{qn}_{rows}")
                        nc.vector.tensor_tensor(t1[sl], e_ps[sl], cs, ALU.mult)
                        nc.vector.tensor_tensor(t2[sl], o_ps[sl], sn, ALU.mult)
                        nc.vector.tensor_tensor(er[sl, osl], t1[sl], t2[sl],
                                                ALU.subtract)
                        t3 = p1.tile([128, QS], F32, tag="rtA", bufs=2,
                                     name=f"t3_{qn}_{rows}")
                        t4 = p1.tile([128, QS], F32, tag="rtB", bufs=2,
                                     name=f"t4_{qn}_{rows}")
                        nc.vector.tensor_tensor(t3[sl], o_ps[sl], cs, ALU.mult)
                        nc.vector.tensor_tensor(t4[sl], e_ps[sl], sn, ALU.mult)
                        nc.vector.tensor_tensor(orr[sl, osl], t3[sl], t4[sl],
                                                ALU.add)

                    if qn % 2 == 0:
                        # rope-output tiles span a token half (2 quarters)
                        ro = {
                            nm: p1.tile([rows, 2 * QS], F8, tag=nm, bufs=2,
                                        name=f"{nm}_{qn // 2}")
                            for nm, rows in (
                                ("er0", 128), ("or0", 128), ("er2", 64),
                                ("or2", 64), ("pq", 128), ("pq2", 128),
                                ("pk1", 128), ("pk2", 64),
                            )
                        }

                    def repack(dst_il, p0, src, s0, nj, slot, c0, cw,
                               hs0=hs0):
                        """src rows s0..s0+4nj (j-major, 4 heads), cols
                        c0..c0+cw -> dst_il partitions p0..p0+nj-1."""
                        nc.sync.dma_start(
                            dst_il[p0:p0 + nj, :, slot, hs0 + c0:hs0 + c0 + cw],
                            src[s0:s0 + 4 * nj, c0:c0 + cw],
                        )

                    # wave 1: T0, T1 -> rope full pair
                    qk_wave([0, 1])
                    rope_pair(qk_ps[0], qk_ps[1], 128, cs0c_t, cs0s_t,
                              ro["er0"], ro["or0"])

                    # wave 2: T2, T3 -> pass copies first (so the k-side
                    # repack isn't stuck behind both rope waves), then rope
                    qk_wave([2, 3])
                    osl = slice(qo, qo + QS)
                    # quarter 0: ACT is idle until this repack completes, so
                    # the pass copies ride there, shortening the DVE chain
                    ceng = nc.scalar.copy if qn == 0 else nc.vector.tensor_copy
                    ceng(ro["pq"][64:128, osl], qk_ps[2][64:128, :])
                    ceng(ro["pq2"][64:96, osl], qk_ps[3][64:96, :])
                    ceng(ro["pk1"][96:128, osl], qk_ps[3][96:128, :])

                    # wave 3: T4 (QK rows only; V is deferred so this
                    # quarter's repack completes as early as possible)
                    qk_wave([4], rows=64)
                    ceng(ro["pk2"][0:64, osl], qk_ps[4][0:64, :])
                    rope_pair(qk_ps[2], qk_ps[3], 64, cs2c_t, cs2s_t,
                              ro["er2"], ro["or2"])

                    # quarter-wise repack into the fp8 DoubleRow layout
                    c0, cw = qo, QS
                    repack(qil, 0, ro["er0"], 0, 24, 0, c0, cw)
                    repack(kil, 0, ro["er0"], 96, 8, 0, c0, cw)
                    repack(qil, 24, ro["or0"], 0, 12, 0, c0, cw)
                    repack(qil, 0, ro["or0"], 48, 12, 1, c0, cw)
                    repack(kil, 24, ro["or0"], 96, 8, 0, c0, cw)
                    repack(kil, 8, ro["er2"], 0, 16, 0, c0, cw)
                    repack(kil, 32, ro["or2"], 0, 4, 0, c0, cw)
                    repack(kil, 0, ro["or2"], 16, 12, 1, c0, cw)
                    repack(qil, 12, ro["pq"], 64, 16, 1, c0, cw)
                    repack(qil, 28, ro["pq2"], 64, 8, 1, c0, cw)
                    repack(kil, 12, ro["pk1"], 96, 8, 1, c0, cw)
                    repack(kil, 20, ro["pk2"], 0, 16, 1, c0, cw)

                    # units made available by this quarter's repack, in
                    # consume order (phase-1 slots are never recycled, so
                    # availability order cannot deadlock the slot ring)
                    for u in p1_units:
                        if u not in emitted and u not in avail:
                            if u[0] <= qn and u[2] // 2 <= qn:
                                avail.append(u)
                    drip(3)
                    emit_v(qn, xq)

                # drain leftover phase-1 units, then finish block (0,0)'s
                # two remaining score units so PV can start at phase-2 entry
                drip(len(avail))
                for kp in range(KT // 2):
                    if (0, 0, kp) not in emitted:
                        emit_unit(0, 0, kp, ps1, "st1", 2, "e2", E2, BF16)

            # ================= phase 2: attention + projection =============
            with (
                tc.tile_pool(name="p2", bufs=1) as p2,
                tc.tile_pool(name="psum2", bufs=1, space="PSUM") as ps2,
            ):
                pending = []
                o_sb_cur = [None]
                parked = {}

                def emit_proj(jq, ct, mode="full"):
                    # mode "A": chunk A only, parked to SBUF (bf16)
                    # mode "B": chunks B+C plus the parked partial, store
                    sl = slice(jq * QS, (jq + 1) * QS)
                    cs = slice(ct * 128, (ct + 1) * 128)
                    o_ps = ps2.tile([128, QS], F32, tag="o_ps", bufs=1,
                                    name=f"ops{jq}_{ct}_{mode}")
                    if mode != "B":
                        nc.tensor.matmul(o_ps[:], wp_t["A"][:, cs], otA[:, sl],
                                         start=True, stop=(mode == "A"))
                    if mode != "A":
                        nc.tensor.matmul(o_ps[:], wp_t["B"][:, cs], otB[:, sl],
                                         start=(mode == "B"), stop=False)
                        nc.tensor.matmul(o_ps[:], wp_t["C"][:, cs], otC[:, sl],
                                         start=False, stop=(mode != "B"))
                    if mode == "B":
                        # fold the parked chunk-A partial back in on the PE
                        nc.tensor.matmul(o_ps[:], iden_t[:], parked[ct][:],
                                         start=False, stop=True)
                    if mode == "A":
                        prk = p2.tile([128, QS], BF16, tag=f"prk{ct}", bufs=1,
                                      name=f"prk{ct}")
                        nc.vector.tensor_copy(prk[:], o_ps[:])
                        parked[ct] = prk
                        return
                    half = ct % 2
                    if half == 0:
                        o_sb_cur[0] = p2.tile([128, 2, QS], F32, tag="o_sb",
                                              bufs=3, name=f"osb{jq}_{ct}")
                    o_sb = o_sb_cur[0]
                    if mode == "B":
                        # ACT is idle after the last exp; drain there
                        nc.scalar.copy(o_sb[:, half, :], o_ps[:])
                    else:
                        nc.vector.tensor_copy(o_sb[:, half, :], o_ps[:])
                    if half == 1 or ct == CK - 1:
                        for i in range(half + 1):
                            r0 = (ct - half + i) * 128
                            nc.sync.dma_start(outT[r0:r0 + 128, sl],
                                              o_sb[:, i, :])

                blocks = [(jq, h) for jq in range(NQ) for h in range(HPG)]

                def emit_units_for(blk):
                    jq, h = blk
                    for kp in range(KT // 2):
                        if (jq, h, kp) not in emitted:
                            emit_unit(jq, h, kp, ps2, "st2", 2, "e2", E2,
                                      BF16)

                emit_units_for(blocks[0])
                for bi, (jq, h) in enumerate(blocks):
                    # software pipeline: exps for the next block first
                    if bi + 1 < len(blocks):
                        emit_units_for(blocks[bi + 1])
                    # PV: out [128 q, 4, 73]; stationary = e chunks
                    pv = ps2.tile([128, 4, 73], F32, tag="pv", bufs=1,
                                  name=f"pv{jq}_{h}")
                    # the PSUM start bit zeroes a whole 2KB bank, which would
                    # wipe sibling q-tile accumulators sharing this bank; so
                    # pre-zero once and accumulate with start=False
                    nc.vector.memset(pv[:], 0.0)
                    # kt-outer so each e_t tile's last read comes early and
                    # its slot frees progressively
                    for kt in range(KT):
                        e_t = e_tiles[(jq, h, kt // 2)]
                        for qt in range(4):
                            c0 = (kt % 2) * QS + qt * 128
                            nc.tensor.matmul(
                                pv[:, qt, :],
                                e_t[:, c0:c0 + 128],
                                v_all[:, kt, h, 0:HD + 1],
                                start=False, stop=(kt == KT - 1),
                                skip_group_check=True,
                            )
                    rec = p2.tile([128, 4], F32, tag="rec", bufs=2,
                                  name=f"rec{jq}_{h}")
                    nc.vector.reciprocal(rec[:], pv[:, :, HD])
                    # each transpose gets a fresh psum tile (its start
                    # bit zeroes the whole bank) and is drained immediately
                    tr_sb = p2.tile([HD, QS], BF16, tag="tr_sb", bufs=2,
                                    name=f"trsb{jq}_{h}")
                    for qt in range(4):
                        onr = p2.tile([128, HD], BF16, tag="onr", bufs=3,
                                      name=f"onr{jq}_{h}_{qt}")
                        nc.vector.tensor_scalar_mul(
                            onr[:], pv[:, qt, 0:HD], rec[:, qt:qt + 1])
                        tr = ps2.tile([HD, 128], BF16, tag="tr", bufs=2,
                                      name=f"tr{jq}_{h}_{qt}")
                        nc.tensor.transpose(tr[:], onr[:], iden_t[:])
                        nc.vector.tensor_copy(
                            tr_sb[:, qt * 128:(qt + 1) * 128], tr[:])
                    sl = slice(jq * QS, (jq + 1) * QS)
                    r0 = h * HD
                    for dst, lo, hi in (
                        (otA, 0, 128), (otB, 128, 256), (otC, 256, 288),
                    ):
                        a, b_ = max(lo, r0), min(hi, r0 + HD)
                        if a < b_:
                            nc.sync.dma_start(
                                dst[a - lo:b_ - lo, sl],
                                tr_sb[a - r0:b_ - r0, :])
                    for _ in range(6):
                        if pending:
                            emit_proj(*pending.pop(0))
                    if h == HPG - 1 and jq < NQ - 1:
                        pending.extend((jq, ct) for ct in range(CK))
                    if jq == NQ - 1 and h == 1:
                        # last-q-chunk tail split: chunk A (heads 0-1 rows)
                        # can project as soon as their o^T rows land
                        pending.extend((NQ - 1, ct, "A") for ct in range(CK))
                while pending:
                    emit_proj(*pending.pop(0))
                for ct in range(CK):
                    emit_proj(NQ - 1, ct, "B")

    bass_rust.generate_event_semaphores(nc)
    return nc


_NC = None


def _get_nc():
    global _NC
    if _NC is None:
        _NC = build_nc()
    return _NC


def _pack_inputs(x, Wqkv, Wproj, bproj):
    bf = ml_dtypes.bfloat16
    cos0, sin0, cos2, sin2 = _host_tables()
    iden = np.eye(128, dtype=bf)

    def qcol(g, h, d):  # q column of head 4g+h dim d
        return (4 * g + h) * HD + d

    def kcol(g, h, d):
        return C + (4 * g + h) * HD + d

    in_maps = []
    for core in range(NCORES):
        b, g = divmod(core, HPG)
        cols = []
        for j in range(24):          # T0: Q_E ; K_E j0..7
            cols += [qcol(g, h, 2 * j) for h in range(4)]
        for j in range(8):
            cols += [kcol(g, h, 2 * j) for h in range(4)]
        for j in range(24):          # T1: Q_O ; K_O j0..7
            cols += [qcol(g, h, 2 * j + 1) for h in range(4)]
        for j in range(8):
            cols += [kcol(g, h, 2 * j + 1) for h in range(4)]
        for j in range(8, 24):       # T2: K_E j8..23 ; Q_P j0..15
            cols += [kcol(g, h, 2 * j) for h in range(4)]
        for j in range(16):
            cols += [qcol(g, h, 48 + j) for h in range(4)]
        for j in range(8, 24):       # T3: K_O j8..23 ; Q_P j16..23 ; K_P j0..7
            cols += [kcol(g, h, 2 * j + 1) for h in range(4)]
        for j in range(16, 24):
            cols += [qcol(g, h, 48 + j) for h in range(4)]
        for j in range(8):
            cols += [kcol(g, h, 48 + j) for h in range(4)]
        for j in range(8, 24):       # T4: K_P j8..23
            cols += [kcol(g, h, 48 + j) for h in range(4)]
        assert len(cols) == QK_ROWS

        cols += list(range(2 * C + 4 * g * HD, 2 * C + (4 * g + 4) * HD))
        wqkv_c = Wqkv[:, cols].astype(bf)                           # [C, 864]
        wp_rows = Wproj[4 * g * HD:(4 * g + 4) * HD, :].astype(bf)  # [288, C]
        wpAB = np.stack([wp_rows[0:128], wp_rows[128:256]], axis=1)
        wpC = np.zeros((33, C), dtype=bf)
        wpC[0:32] = wp_rows[256:288]
        wpC[32] = (bproj if g == 0 else np.zeros_like(bproj)).astype(bf)
        cs0 = np.stack([cos0, sin0], axis=1)    # [128, 2, N]
        cs2 = np.stack([cos2, sin2], axis=1)    # [64, 2, N]

        in_maps.append({
            "xT": np.ascontiguousarray(x[b].T).astype(bf).view(np.uint16),
            "wqkv": np.ascontiguousarray(wqkv_c).view(np.uint16),
            "wpAB": np.ascontiguousarray(wpAB).view(np.uint16),
            "wpC": wpC.view(np.uint16),
            "cs0": np.ascontiguousarray(cs0).view(np.uint16),
            "cs2": np.ascontiguousarray(cs2).view(np.uint16),
            "iden": iden.view(np.uint16),
        })
    return in_maps


def kernel(x, Wqkv, Wproj, bproj, T, H, W):
    x = np.asarray(x, dtype=np.float32)
    Wqkv = np.asarray(Wqkv, dtype=np.float32)
    Wproj = np.asarray(Wproj, dtype=np.float32)
    bproj = np.asarray(bproj, dtype=np.float32)
    assert x.shape == (B, N, C) and Wqkv.shape == (C, 3 * C)
    assert (int(T), int(H), int(W)) == (GT, GH, GW)

    nc = _get_nc()
    in_maps = _pack_inputs(x, Wqkv, Wproj, bproj)
    res = run_bass_kernel_spmd(nc, in_maps, core_ids=list(range(NCORES)))
    out = np.zeros((B, N, C), dtype=np.float32)
    for core in range(NCORES):
        r = np.asarray(res.results[core]["outT"])
        if r.dtype != np.float32:
            r = r.view(ml_dtypes.bfloat16).astype(np.float32)
        out[core // HPG] += r.T
    return out


# revision 72
# speedup vs baseline: 1.0136x; 1.0027x over previous
"""Trainium2 Bass kernel for nn_Attention_79224966742132.

Dense transformer attention block: QKV projection + axial RoPE + SDPA +
output projection, for x (2, 2048, 1152), 16 heads of dim 72.

Sharding (8 cores): data-parallel over batch (2) x tensor-parallel over
head groups (4 heads/core). Each core computes QKV for its 4 heads from
the full x[b], applies RoPE, runs attention, and produces a partial
output projection (row-parallel Wproj); the host sums the 4 partials per
batch element. The projection bias rides on the g==0 core of each batch.

Engine plan (per the TRN2 cost model):
 - all GEMM inputs bf16 (1 cycle/row); scores q/k quantized to fp8e4 and
   run in DoubleRow perf mode (0.5 cycles/row) with the 72-dim
   contraction packed as [36 partitions x 2 slots].
 - exp of all N^2 scores is the critical ~130us ACT chain; score->exp
   units are interleaved into phase 1 (capped by the e_t inventory E so
   the in-order PE queue can never block on a PV-gated slot) so ACT
   starts early and never runs dry afterwards.
 - PV is "flipped": out [128 q, 73] with the exp tile as stationary, so
   each matmul streams only F=73 rows; a ones column in V yields the
   softmax denominator; normalize via per-partition tensor_scalar; PE
   transposes (bf16) restore o^T for the row-parallel projection.
 - projection contraction packed to [128,128,33] chunks with the bias as
   a ones-row in the last chunk (o^T rows staged via aligned DVE copy +
   partition-arbitrary SBUF->SBUF DMA).
"""
import math
import os
import sys

# The device path needs the axon/neuron jax platform; if a harness pinned
# JAX_PLATFORMS=cpu (common for running jax references) and jax is not yet
# imported, restore platform auto-detection.
if "jax" not in sys.modules:
    _jp = os.environ.get("JAX_PLATFORMS")
    if _jp and "axon" not in _jp and "neuron" not in _jp:
        del os.environ["JAX_PLATFORMS"]

import numpy as np
import ml_dtypes

import bass_rust
import concourse.bass as bass
import concourse.mybir as mybir
import concourse.tile as tile
from concourse.bass_utils import run_bass_kernel_spmd

F32 = mybir.dt.float32
BF16 = mybir.dt.bfloat16
F8 = mybir.dt.float8e4
AF = mybir.ActivationFunctionType
ALU = mybir.AluOpType
DR = mybir.MatmulPerfMode.DoubleRow

B = 2
N = 2048          # tokens = T*H*W = 8*16*16
C = 1152
NH = 16
HD = 72
HPG = 4           # heads per core
NCORES = 8
GT, GH, GW = 8, 16, 16
SCALE = 1.0 / math.sqrt(HD)

NQ = 4            # token quarters
QS = N // NQ      # 512
KT = N // 128     # 16 k-tiles
CK = C // 128     # 9 contraction chunks of x
# e_t inventory: phase-1 units get dedicated slots (E1 tiles, no recycling,
# so emission order is unconstrained); phase-2 units recycle E2 slots in
# strict consumption order.
E1 = 40
E2 = 16

# phase-1 QK row tiles: T0..T3 are 128 rows, T4 is 64.
# Within a 96-row block rows are j-major: row = 4*j + h.
#  T0 = [Q_E(96); K_E j0..7]   T1 = [Q_O(96); K_O j0..7]
#  T2 = [K_E j8..23; Q_P j0..15]
#  T3 = [K_O j8..23; Q_P j16..23; K_P j0..7]
#  T4 = [K_P j8..23]
QK_ROWS = 576


def _axis_freqs(n: int) -> np.ndarray:
    base = np.linspace(1.0, 128.0, 8, dtype=np.float64) * np.pi  # MAX_FREQ/2
    pos = np.linspace(-1.0, 1.0, n, dtype=np.float64)
    return pos[:, None] * base[None, :]


def _freq24() -> np.ndarray:
    """per-token frequency of rotary pair j (24 pairs) -> (N, 24)."""
    f = np.zeros((GT, GH, GW, 24), dtype=np.float64)
    f[..., 0:8] = _axis_freqs(GT)[:, None, None, :]
    f[..., 8:16] = _axis_freqs(GH)[None, :, None, :]
    f[..., 16:24] = _axis_freqs(GW)[None, None, :, :]
    return f.reshape(N, 24)


def _host_tables():
    f24 = _freq24()                      # (N, 24)
    cos24 = np.cos(f24).T                # (24, N) float64
    sin24 = np.sin(f24).T
    # cs0: rows r<96: j=r//4 ; rows 96..127: j=(r-96)//4
    j0 = np.concatenate([np.arange(96) // 4, np.arange(32) // 4])
    # cs2: rows 0..63: j = 8 + r//4
    j2 = 8 + np.arange(64) // 4
    bf = ml_dtypes.bfloat16
    return (
        cos24[j0].astype(bf), sin24[j0].astype(bf),
        cos24[j2].astype(bf), sin24[j2].astype(bf),
    )


def build_nc() -> bass.Bass:
    nc = bass.Bass()
    xT = nc.dram_tensor("xT", [C, N], BF16, kind="ExternalInput")
    # wqkv = [wqk rows (576) ; wv cols (288)] fused per contraction chunk
    wqkv = nc.dram_tensor("wqkv", [C, QK_ROWS + HPG * HD], BF16,
                          kind="ExternalInput")
    wpAB = nc.dram_tensor("wpAB", [128, 2, C], BF16, kind="ExternalInput")
    wpC = nc.dram_tensor("wpC", [33, C], BF16, kind="ExternalInput")
    cs0 = nc.dram_tensor("cs0", [128, 2, N], BF16, kind="ExternalInput")
    cs2 = nc.dram_tensor("cs2", [64, 2, N], BF16, kind="ExternalInput")
    iden = nc.dram_tensor("iden", [128, 128], BF16, kind="ExternalInput")
    outT = nc.dram_tensor("outT", [C, N], F32, kind="ExternalOutput")

    with tile.TileContext(nc) as tc:
        with tc.tile_pool(name="persist", bufs=1) as pp:
            # fp8 q/k in DoubleRow block layout: [36, head, slot, token]
            qil = pp.tile([36, HPG, 2, N], F8, name="qil")
            kil = pp.tile([36, HPG, 2, N], F8, name="kil")
            # v with ones column: [token128, ktile, head, 80] (73 used)
            v_all = pp.tile([128, KT, HPG, 80], BF16, name="v_all")
            # packed o^T rows for the projection: 289 rows in 128+128+33
            otA = pp.tile([128, N], BF16, name="otA")
            otB = pp.tile([128, N], BF16, name="otB")
            otC = pp.tile([33, N], BF16, name="otC")
            iden_t = pp.tile([128, 128], BF16, name="iden_t")
            cs0_t = pp.tile([128, 2, N], BF16, name="cs0_t")
            cs2_t = pp.tile([64, 2, N], BF16, name="cs2_t")
            wpAB_t = pp.tile([128, 2, C], BF16, name="wpAB_t")
            wpC_t = pp.tile([33, C], BF16, name="wpC_t")
            cs0c_t, cs0s_t = cs0_t[:, 0], cs0_t[:, 1]
            cs2c_t, cs2s_t = cs2_t[:, 0], cs2_t[:, 1]
            wp_t = {"A": wpAB_t[:, 0], "B": wpAB_t[:, 1], "C": wpC_t[:]}
            nc.vector.memset(v_all[:, :, :, HD], 1.0)
            nc.vector.memset(otC[32:33, :], 1.0)

            # ---- exp units -----------------------------------------------
            # unit = (jq, h, kp): scores for q-chunk jq (512 q) against
            # k-tiles 2kp, 2kp+1 -> one exp of [128, 1024] -> e_t (bf16).
            e_tiles = {}
            emitted = set()

            def emit_unit(jq, h, kp, ps_pool, st_tag, st_bufs, e_tag, e_bufs,
                          e_dt):
                st = ps_pool.tile([128, 2 * QS], F32, tag=st_tag,
                                  bufs=st_bufs, name=f"st_{jq}_{h}_{kp}")
                for i in range(2):
                    kt = 2 * kp + i
                    nc.tensor.matmul(
                        st[:, i * QS:(i + 1) * QS],
                        kil[:, h, :, kt * 128:(kt + 1) * 128],
                        qil[:, h, :, jq * QS:(jq + 1) * QS],
                        start=True, stop=True, perf_mode=DR,
                    )
                e_t = pp.tile([128, 2 * QS], e_dt, tag=e_tag, bufs=e_bufs,
                              name=f"e_{jq}_{h}_{kp}")
                nc.scalar.activation(e_t[:], st[:], AF.Exp, scale=SCALE)
                e_tiles[(jq, h, kp)] = e_t
                emitted.add((jq, h, kp))

            consume_order = [
                (jq, h, kp) for jq in range(NQ) for h in range(HPG)
                for kp in range(KT // 2)
            ]
            # phase-1 units: spread across jq<=2 blocks (kp<=2 each, plus
            # kp=3 for jq0) so every phase-2 block retains >=4 live exp units
            # to hide its PV->norm->drain latency chain behind ACT work
            p1_units = [u for u in consume_order
                        if (u[0] <= 2 and u[2] <= 2)
                        or (u[0] == 0 and u[2] == 3)][:E1]
            avail = []

            def drip(n):
                for _ in range(n):
                    if not avail:
                        return
                    jq, h, kp = avail.pop(0)
                    emit_unit(jq, h, kp, ps1_ref[0], "st1", 2, "e1", E1, F8)

            # ================= phase 1: QKV + RoPE + repack ================
            with (
                tc.tile_pool(name="p1", bufs=1) as p1,
                tc.tile_pool(name="psum1", bufs=1, space="PSUM") as ps1,
            ):
                ps1_ref = [ps1]
                wqkv_t = [p1.tile([128, QK_ROWS + HPG * HD], BF16,
                                  name=f"wqkv{k}") for k in range(CK)]
                wqk_t = [t[:, 0:QK_ROWS] for t in wqkv_t]
                wv_t = [t[:, QK_ROWS:] for t in wqkv_t]

                ro = {}  # rope-output tiles, allocated per token-half

                def emit_v(vq, vxq):
                    for tt in range(4):
                        v_ps = ps1.tile([128, QS], F32, tag="qk",
                                        bufs=4, name=f"vps{vq}_{tt}")
                        for k in range(CK):
                            nc.tensor.matmul(
                                v_ps[:, 0:HPG * HD],
                                vxq[k][:, tt * 128:(tt + 1) * 128],
                                wv_t[k][:],
                                start=(k == 0), stop=(k == CK - 1),
                            )
                            if k % 4 == 1:
                                drip(1)
                        nc.vector.tensor_copy(
                            v_all[:, vq * 4 + tt, :, 0:HD],
                            v_ps[:, 0:HPG * HD]
                            .rearrange("p (h d) -> p h d", h=HPG),
                        )

                for qn in range(NQ):
                    ts0 = qn * QS
                    hs0 = (qn // 2) * 2 * QS   # token base of this half
                    qo = (qn % 2) * QS         # quarter offset within half
                    if qn % 2 == 0:
                        # x loads at half-token granularity: half the DMA
                        # count on the serial DGE front-end, and the odd
                        # quarters never wait on x at all
                        xh = [p1.tile([128, 2 * QS], BF16, tag=f"xq{k}",
                                      bufs=1, name=f"xq{k}_{qn}")
                              for k in range(CK)]
                    xq = [t[:, qo:qo + QS] for t in xh]
                    for k in range(CK):
                        if qn % 2 == 0:
                            nc.sync.dma_start(
                                xh[k][:],
                                xT[k * 128:(k + 1) * 128, hs0:hs0 + 2 * QS])
                        if qn == 0:
                            nc.sync.dma_start(
                                wqkv_t[k][:], wqkv[k * 128:(k + 1) * 128, :])
                            if k == 1:
                                # cs tables ride behind the first two chunk
                                # pairs so the PE can start immediately
                                nc.sync.dma_start(cs0_t[:], cs0.ap())
                                nc.sync.dma_start(cs2_t[:], cs2.ap())
                    if qn == 0:
                        nc.sync.dma_start(wpAB_t[:], wpAB.ap())
                        nc.sync.dma_start(wpC_t[:], wpC.ap())
                        nc.sync.dma_start(iden_t[:], iden.ap())

                    qk_ps = {}

                    def qk_wave(tiles, rows=128, qn=qn, xq=xq, qk_ps=qk_ps):
                        for t in tiles:
                            qk_ps[t] = ps1.tile(
                                [128, QS], F32, tag="qk", bufs=4,
                                name=f"qk{t}_{qn}")
                        for k in range(CK):
                            for t in tiles:
                                nc.tensor.matmul(
                                    qk_ps[t][0:rows, :],
                                    wqk_t[k][:, t * 128:t * 128 + rows],
                                    xq[k][:],
                                    start=(k == 0), stop=(k == CK - 1),
                                )
                            if k % 2 == 1:
                                # drip sites spaced ~1us: matches ACT pace
                                drip(1)

                    def rope_pair(e_ps, o_ps, rows, cosA, sinA, er, orr,
                                  qn=qn, ts0=ts0, qo=qo):
                        """er = e*cos - o*sin ; or = o*cos + e*sin (fp8 out)"""
                        sl = slice(0, rows)
                        cs = cosA[sl, ts0:ts0 + QS]
                        sn = sinA[sl, ts0:ts0 + QS]
                        osl = slice(qo, qo + QS)
                        t1 = p1.tile([128, QS], F32, tag="rtA", bufs=2,
                                     name=f"t1_{qn}_{rows}")
                        t2 = p1.tile([128, QS], F32, tag="rtB", bufs=2,
                                     name=f"t2_{qn}_{rows}")
                        nc.vector.tensor_tensor(t1[sl], e_ps[sl], cs, ALU.mult)
                        nc.vector.tensor_tensor(t2[sl], o_ps[sl], sn, ALU.mult)
                        nc.vector.tensor_tensor(er[sl, osl], t1[sl], t2[sl],
                                                ALU.subtract)
                        t3 = p1.tile([128, QS], F32, tag="rtA", bufs=2,
                                     name=f"t3_{qn}_{rows}")
                        t4 = p1.tile([128, QS], F32, tag="rtB", bufs=2,
                                     name=f"t4_{qn}_{rows}")
                        nc.vector.tensor_tensor(t3[sl], o_ps[sl], cs, ALU.mult)
                        nc.vector.tensor_tensor(t4[sl], e_ps[sl], sn, ALU.mult)
                        nc.vector.tensor_tensor(orr[sl, osl], t3[sl], t4[sl],
                                                ALU.add)

                    if qn % 2 == 0:
                        # rope-output tiles span a token half (2 quarters)
                        ro = {
                            nm: p1.tile([rows, 2 * QS], F8, tag=nm, bufs=2,
                                        name=f"{nm}_{qn // 2}")
                            for nm, rows in (
                                ("er0", 128), ("or0", 128), ("er2", 64),
                                ("or2", 64), ("pq", 128), ("pq2", 128),
                                ("pk1", 128), ("pk2", 64),
                            )
                        }

                    def repack(dst_il, p0, src, s0, nj, slot, c0, cw,
                               hs0=hs0):
                        """src rows s0..s0+4nj (j-major, 4 heads), cols
                        c0..c0+cw -> dst_il partitions p0..p0+nj-1."""
                        nc.sync.dma_start(
                            dst_il[p0:p0 + nj, :, slot, hs0 + c0:hs0 + c0 + cw],
                            src[s0:s0 + 4 * nj, c0:c0 + cw],
                        )

                    # wave 1: T0, T1 -> rope full pair
                    qk_wave([0, 1])
                    rope_pair(qk_ps[0], qk_ps[1], 128, cs0c_t, cs0s_t,
                              ro["er0"], ro["or0"])

                    # wave 2: T2, T3 -> pass copies first (so the k-side
                    # repack isn't stuck behind both rope waves), then rope
                    qk_wave([2, 3])
                    osl = slice(qo, qo + QS)
                    # quarter 0: ACT is idle until this repack completes, so
                    # the pass copies ride there, shortening the DVE chain
                    ceng = nc.scalar.copy if qn == 0 else nc.vector.tensor_copy
                    ceng(ro["pq"][64:128, osl], qk_ps[2][64:128, :])
                    ceng(ro["pq2"][64:96, osl], qk_ps[3][64:96, :])
                    ceng(ro["pk1"][96:128, osl], qk_ps[3][96:128, :])

                    # wave 3: T4 (QK rows only; V is deferred so this
                    # quarter's repack completes as early as possible)
                    qk_wave([4], rows=64)
                    ceng(ro["pk2"][0:64, osl], qk_ps[4][0:64, :])
                    rope_pair(qk_ps[2], qk_ps[3], 64, cs2c_t, cs2s_t,
                              ro["er2"], ro["or2"])

                    # quarter-wise repack into the fp8 DoubleRow layout
                    c0, cw = qo, QS
                    repack(qil, 0, ro["er0"], 0, 24, 0, c0, cw)
                    repack(kil, 0, ro["er0"], 96, 8, 0, c0, cw)
                    repack(qil, 24, ro["or0"], 0, 12, 0, c0, cw)
                    repack(qil, 0, ro["or0"], 48, 12, 1, c0, cw)
                    repack(kil, 24, ro["or0"], 96, 8, 0, c0, cw)
                    repack(kil, 8, ro["er2"], 0, 16, 0, c0, cw)
                    repack(kil, 32, ro["or2"], 0, 4, 0, c0, cw)
                    repack(kil, 0, ro["or2"], 16, 12, 1, c0, cw)
                    repack(qil, 12, ro["pq"], 64, 16, 1, c0, cw)
                    repack(qil, 28, ro["pq2"], 64, 8, 1, c0, cw)
                    repack(kil, 12, ro["pk1"], 96, 8, 1, c0, cw)
                    repack(kil, 20, ro["pk2"], 0, 16, 1, c0, cw)

                    # units made available by this quarter's repack, in
                    # consume order (phase-1 slots are never recycled, so
                    # availability order cannot deadlock the slot ring)
                    for u in p1_units:
                        if u not in emitted and u not in avail:
                            if u[0] <= qn and u[2] // 2 <= qn:
                                avail.append(u)
                    drip(3)
                    emit_v(qn, xq)

                # drain leftover phase-1 units
                drip(len(avail))

            # ================= phase 2: attention + projection =============
            with (
                tc.tile_pool(name="p2", bufs=1) as p2,
                tc.tile_pool(name="psum2", bufs=1, space="PSUM") as ps2,
            ):
                pending = []
                o_sb_cur = [None]
                parked = {}

                def emit_proj(jq, ct, mode="full"):
                    # mode "A": chunk A only, parked to SBUF (bf16)
                    # mode "B": chunks B+C plus the parked partial, store
                    sl = slice(jq * QS, (jq + 1) * QS)
                    cs = slice(ct * 128, (ct + 1) * 128)
                    o_ps = ps2.tile([128, QS], F32, tag="o_ps", bufs=1,
                                    name=f"ops{jq}_{ct}_{mode}")
                    if mode != "B":
                        nc.tensor.matmul(o_ps[:], wp_t["A"][:, cs], otA[:, sl],
                                         start=True, stop=(mode == "A"))
                    if mode != "A":
                        nc.tensor.matmul(o_ps[:], wp_t["B"][:, cs], otB[:, sl],
                                         start=(mode == "B"), stop=False)
                        nc.tensor.matmul(o_ps[:], wp_t["C"][:, cs], otC[:, sl],
                                         start=False, stop=(mode != "B"))
                    if mode == "B":
                        # fold the parked chunk-A partial back in on the PE
                        nc.tensor.matmul(o_ps[:], iden_t[:], parked[ct][:],
                                         start=False, stop=True)
                    if mode == "A":
                        prk = p2.tile([128, QS], BF16, tag=f"prk{ct}", bufs=1,
                                      name=f"prk{ct}")
                        nc.vector.tensor_copy(prk[:], o_ps[:])
                        parked[ct] = prk
                        return
                    half = ct % 2
                    if half == 0:
                        o_sb_cur[0] = p2.tile([128, 2, QS], F32, tag="o_sb",
                                              bufs=3, name=f"osb{jq}_{ct}")
                    o_sb = o_sb_cur[0]
                    if mode == "B":
                        # ACT is idle after the last exp; drain there
                        nc.scalar.copy(o_sb[:, half, :], o_ps[:])
                    else:
                        nc.vector.tensor_copy(o_sb[:, half, :], o_ps[:])
                    if half == 1 or ct == CK - 1:
                        for i in range(half + 1):
                            r0 = (ct - half + i) * 128
                            nc.sync.dma_start(outT[r0:r0 + 128, sl],
                                              o_sb[:, i, :])

                blocks = [(jq, h) for jq in range(NQ) for h in range(HPG)]

                def emit_units_for(blk):
                    jq, h = blk
                    for kp in range(KT // 2):
                        if (jq, h, kp) not in emitted:
                            emit_unit(jq, h, kp, ps2, "st2", 2, "e2", E2,
                                      BF16)

                emit_units_for(blocks[0])
                for bi, (jq, h) in enumerate(blocks):
                    # software pipeline: exps for the next block first
                    if bi + 1 < len(blocks):
                        emit_units_for(blocks[bi + 1])
                    # PV: out [128 q, 4, 73]; stationary = e chunks
                    pv = ps2.tile([128, 4, 73], F32, tag="pv", bufs=1,
                                  name=f"pv{jq}_{h}")
                    # the PSUM start bit zeroes a whole 2KB bank, which would
                    # wipe sibling q-tile accumulators sharing this bank; so
                    # pre-zero once and accumulate with start=False
                    nc.vector.memset(pv[:], 0.0)
                    # kt-outer so each e_t tile's last read comes early and
                    # its slot frees progressively
                    for kt in range(KT):
                        e_t = e_tiles[(jq, h, kt // 2)]
                        for qt in range(4):
                            c0 = (kt % 2) * QS + qt * 128
                            nc.tensor.matmul(
                                pv[:, qt, :],
                                e_t[:, c0:c0 + 128],
                                v_all[:, kt, h, 0:HD + 1],
                                start=False, stop=(kt == KT - 1),
                                skip_group_check=True,
                            )
                    rec = p2.tile([128, 4], F32, tag="rec", bufs=2,
                                  name=f"rec{jq}_{h}")
                    nc.vector.reciprocal(rec[:], pv[:, :, HD])
                    # each transpose gets a fresh psum tile (its start
                    # bit zeroes the whole bank) and is drained immediately
                    tr_sb = p2.tile([HD, QS], BF16, tag="tr_sb", bufs=2,
                                    name=f"trsb{jq}_{h}")
                    for qt in range(4):
                        onr = p2.tile([128, HD], BF16, tag="onr", bufs=3,
                                      name=f"onr{jq}_{h}_{qt}")
                        nc.vector.tensor_scalar_mul(
                            onr[:], pv[:, qt, 0:HD], rec[:, qt:qt + 1])
                        tr = ps2.tile([HD, 128], BF16, tag="tr", bufs=2,
                                      name=f"tr{jq}_{h}_{qt}")
                        nc.tensor.transpose(tr[:], onr[:], iden_t[:])
                        nc.vector.tensor_copy(
                            tr_sb[:, qt * 128:(qt + 1) * 128], tr[:])
                    sl = slice(jq * QS, (jq + 1) * QS)
                    r0 = h * HD
                    for dst, lo, hi in (
                        (otA, 0, 128), (otB, 128, 256), (otC, 256, 288),
                    ):
                        a, b_ = max(lo, r0), min(hi, r0 + HD)
                        if a < b_:
                            nc.sync.dma_start(
                                dst[a - lo:b_ - lo, sl],
                                tr_sb[a - r0:b_ - r0, :])
                    for _ in range(6):
                        if pending:
                            emit_proj(*pending.pop(0))
                    if h == HPG - 1 and jq < NQ - 1:
                        pending.extend((jq, ct) for ct in range(CK))
                    if jq == NQ - 1 and h == 1:
                        # last-q-chunk tail split: chunk A (heads 0-1 rows)
                        # can project as soon as their o^T rows land
                        pending.extend((NQ - 1, ct, "A") for ct in range(CK))
                while pending:
                    emit_proj(*pending.pop(0))
                for ct in range(CK):
                    emit_proj(NQ - 1, ct, "B")

    bass_rust.generate_event_semaphores(nc)
    return nc


_NC = None


def _get_nc():
    global _NC
    if _NC is None:
        _NC = build_nc()
    return _NC


def _pack_inputs(x, Wqkv, Wproj, bproj):
    bf = ml_dtypes.bfloat16
    cos0, sin0, cos2, sin2 = _host_tables()
    iden = np.eye(128, dtype=bf)

    def qcol(g, h, d):  # q column of head 4g+h dim d
        return (4 * g + h) * HD + d

    def kcol(g, h, d):
        return C + (4 * g + h) * HD + d

    in_maps = []
    for core in range(NCORES):
        b, g = divmod(core, HPG)
        cols = []
        for j in range(24):          # T0: Q_E ; K_E j0..7
            cols += [qcol(g, h, 2 * j) for h in range(4)]
        for j in range(8):
            cols += [kcol(g, h, 2 * j) for h in range(4)]
        for j in range(24):          # T1: Q_O ; K_O j0..7
            cols += [qcol(g, h, 2 * j + 1) for h in range(4)]
        for j in range(8):
            cols += [kcol(g, h, 2 * j + 1) for h in range(4)]
        for j in range(8, 24):       # T2: K_E j8..23 ; Q_P j0..15
            cols += [kcol(g, h, 2 * j) for h in range(4)]
        for j in range(16):
            cols += [qcol(g, h, 48 + j) for h in range(4)]
        for j in range(8, 24):       # T3: K_O j8..23 ; Q_P j16..23 ; K_P j0..7
            cols += [kcol(g, h, 2 * j + 1) for h in range(4)]
        for j in range(16, 24):
            cols += [qcol(g, h, 48 + j) for h in range(4)]
        for j in range(8):
            cols += [kcol(g, h, 48 + j) for h in range(4)]
        for j in range(8, 24):       # T4: K_P j8..23
            cols += [kcol(g, h, 48 + j) for h in range(4)]
        assert len(cols) == QK_ROWS

        cols += list(range(2 * C + 4 * g * HD, 2 * C + (4 * g + 4) * HD))
        wqkv_c = Wqkv[:, cols].astype(bf)                           # [C, 864]
        wp_rows = Wproj[4 * g * HD:(4 * g + 4) * HD, :].astype(bf)  # [288, C]
        wpAB = np.stack([wp_rows[0:128], wp_rows[128:256]], axis=1)
        wpC = np.zeros((33, C), dtype=bf)
        wpC[0:32] = wp_rows[256:288]
        wpC[32] = (bproj if g == 0 else np.zeros_like(bproj)).astype(bf)
        cs0 = np.stack([cos0, sin0], axis=1)    # [128, 2, N]
        cs2 = np.stack([cos2, sin2], axis=1)    # [64, 2, N]

        in_maps.append({
            "xT": np.ascontiguousarray(x[b].T).astype(bf).view(np.uint16),
            "wqkv": np.ascontiguousarray(wqkv_c).view(np.uint16),
            "wpAB": np.ascontiguousarray(wpAB).view(np.uint16),
            "wpC": wpC.view(np.uint16),
            "cs0": np.ascontiguousarray(cs0).view(np.uint16),
            "cs2": np.ascontiguousarray(cs2).view(np.uint16),
            "iden": iden.view(np.uint16),
        })
    return in_maps


def kernel(x, Wqkv, Wproj, bproj, T, H, W):
    x = np.asarray(x, dtype=np.float32)
    Wqkv = np.asarray(Wqkv, dtype=np.float32)
    Wproj = np.asarray(Wproj, dtype=np.float32)
    bproj = np.asarray(bproj, dtype=np.float32)
    assert x.shape == (B, N, C) and Wqkv.shape == (C, 3 * C)
    assert (int(T), int(H), int(W)) == (GT, GH, GW)

    nc = _get_nc()
    in_maps = _pack_inputs(x, Wqkv, Wproj, bproj)
    res = run_bass_kernel_spmd(nc, in_maps, core_ids=list(range(NCORES)))
    out = np.zeros((B, N, C), dtype=np.float32)
    for core in range(NCORES):
        r = np.asarray(res.results[core]["outT"])
        if r.dtype != np.float32:
            r = r.view(ml_dtypes.bfloat16).astype(np.float32)
        out[core // HPG] += r.T
    return out
